# revision 6
# baseline (speedup 1.0000x reference)
"""Two-layer GAT (PyG GATConv math) on 8 Trainium2 NeuronCores via Bass/Tile.

v2: batched edge gathers via InstDMAGatherAnt (dma_gather) + bf16 table.

Sharding: nodes split into 8 contiguous ranges of 12500; each core aggregates
the in-edges of its own nodes. Within a core, nodes are grouped into 128-node
windows ordered to balance per-source-quadrant degrees (argmax/max/total sort).

Per layer:
  node phase  - h = x @ W (bf16 PE matmuls) and attention scores es/ed,
                written as 256-byte bf16 table rows [h(64) | es(8) | pad(56)];
                AllGather replicates the 100352-row table to every core.
  edge phase  - the global table is split into 4 quadrants of 25088 rows so
                indices fit int16 (dma_gather's index dtype). For each group
                of GW windows x quadrant, ONE dma_gather fetches all slots
                (token j -> partition j%128 = dst lane, column j//128).
                Per window: z=leaky(es+ed), exp (ACT), partial den/num
                reduced and accumulated into persistent f32 buffers; padded
                slots point at a phantom row with es=-1e30 so exp()=0.
"""
import sys

sys.path.insert(0, "/opt/trn_rl_repo")

import numpy as np

import concourse.bass as bass
import concourse.bacc as bacc
import concourse.tile as tile
from concourse import mybir
from concourse.bass import AP, IndirectOffsetOnAxis
from concourse.masks import make_identity

F32 = mybir.dt.float32
BF16 = mybir.dt.bfloat16
I16 = mybir.dt.int16
AX = mybir.AxisListType.X
OP = mybir.AluOpType
AF = mybir.ActivationFunctionType

N = 100_000
F_IN = 512
H1, FH1 = 8, 8
D1 = H1 * FH1          # 64
C = 64
NCORES = 8
NLR = N // NCORES      # 12500 real nodes per core
PW = 128
NWIN = (NLR + PW - 1) // PW   # 98
NL = NWIN * PW         # 12544
GT = NCORES * NL       # 100352 table rows
ELEM = 128             # bf16 elements per table row (256 B)
NQ = 4
QN = N // NQ           # 25000 source nodes per quadrant
QR = 2 * NL            # 25088 table rows per quadrant
PHANTOM = NLR          # quadrant-local phantom row index (12500)
NEG = -1.0e30
GW = 7                 # windows per gather group
NGRP = NWIN // GW      # 14
TOKCHUNK = 1024        # max tokens per packed dma_gather call (64 desc/engine)
SB = 7                 # windows per node-phase staging batch
XB = 2                 # windows per x-load batch


# ---------------------------------------------------------------- host planning
def _assign_cores(src, dst):
    """Greedy: assign nodes to the 4 source-quadrants so each dst's in-edges
    split evenly across quadrants (cuts slot-grid padding), then split each
    quadrant into its 2 cores."""
    order = np.argsort(src, kind="stable")
    s_s, d_s = src[order], dst[order]
    outdeg = np.bincount(s_s, minlength=N)
    starts = np.concatenate([[0], np.cumsum(outdeg)])
    cnt = np.zeros((N, NQ), np.int32)
    cap = np.full(NQ, QN, np.int64)
    qa = np.full(N, -1, np.int8)
    rng = np.random.default_rng(0)
    for v in rng.permutation(N):
        dv = d_s[starts[v] : starts[v + 1]]
        sc = cnt[dv, :].sum(axis=0).astype(np.int64)
        sc = np.where(cap > 0, sc, 1 << 60)
        q = int(np.argmin(sc))
        qa[v] = q
        cap[q] -= 1
        cnt[dv, q] += 1
    core_of = np.empty(N, np.int64)
    for q in range(NQ):
        nodes = np.where(qa == q)[0]
        core_of[nodes[:NLR]] = 2 * q
        core_of[nodes[NLR:]] = 2 * q + 1
    return core_of, [np.where(core_of == c)[0] for c in range(NCORES)]


def _plan(edge_index):
    src = np.concatenate([edge_index[0], np.arange(N)]).astype(np.int64)
    dst = np.concatenate([edge_index[1], np.arange(N)]).astype(np.int64)
    core_of, nodes_c = _assign_cores(src, dst)
    loc = np.empty(N, np.int64)
    for c in range(NCORES):
        loc[nodes_c[c]] = np.arange(NLR)
    owner = core_of[dst]

    per_core, orders, posmaps, degqs = [], [], [], []
    for c in range(NCORES):
        m = owner == c
        s_c, d_c = src[m], loc[dst[m]]
        q_c = core_of[s_c] // 2
        degq = np.zeros((NLR, NQ), np.int64)
        for qq in range(NQ):
            degq[:, qq] = np.bincount(d_c[q_c == qq], minlength=NLR)
        order = np.lexsort((-degq.sum(1), -degq.max(1), degq.argmax(1)))
        posmap = np.empty(NLR, dtype=np.int64)
        posmap[order] = np.arange(NLR)
        per_core.append((s_c, d_c, q_c))
        orders.append(order)
        posmaps.append(posmap)
        degqs.append(degq)

    # shared per-window per-quadrant slot counts
    Kq = np.zeros((NWIN, NQ), np.int64)
    for c in range(NCORES):
        dp = np.zeros((NL, NQ), np.int64)
        dp[:NLR] = degqs[c][orders[c]]
        Kq = np.maximum(Kq, dp.reshape(NWIN, PW, NQ).max(axis=1))
    Kq = np.maximum(Kq, 1)

    # call layout: for g in groups, for q in quadrants: cols[g][q] columns
    cols = np.zeros((NGRP, NQ), np.int64)
    for g in range(NGRP):
        cols[g] = Kq[g * GW : (g + 1) * GW].sum(axis=0)
    # column start of (w, q) inside the whole concatenated stream
    callbase = np.zeros((NGRP, NQ), np.int64)  # column base of call (g, q)
    acc = 0
    for g in range(NGRP):
        for q in range(NQ):
            callbase[g, q] = acc
            acc += cols[g, q]
    totcols = acc
    colstart = np.zeros((NWIN, NQ), np.int64)  # column start of (w, q) block
    for g in range(NGRP):
        for q in range(NQ):
            cw = callbase[g, q]
            for w in range(g * GW, (g + 1) * GW):
                colstart[w, q] = cw
                cw += Kq[w, q]

    # per-core token streams
    idx_streams = []
    for c in range(NCORES):
        s_c, d_c, q_c = per_core[c]
        pos = posmaps[c][d_c]
        cv = core_of[s_c]
        tmp = np.empty(len(s_c), dtype=np.int64)
        for o in range(NCORES):
            mo = cv == o
            tmp[mo] = posmaps[o][loc[s_c[mo]]]
        srow_local = (cv % 2) * NL + tmp
        # k = rank within (q, pos)
        key = q_c * NL + pos
        ordk = np.argsort(key, kind="stable")
        key_s = key[ordk]
        cnt = np.bincount(key_s, minlength=NQ * NL)
        starts = np.concatenate([[0], np.cumsum(cnt)])[:-1]
        k_of = np.arange(len(key_s)) - starts[key_s]
        pos_s = pos[ordk]
        q_s = q_c[ordk]
        srow_s = srow_local[ordk]
        w_s = pos_s >> 7
        p_s = pos_s & 127
        col = colstart[w_s, q_s] + k_of
        tokpos = col * 128 + p_s
        stream = np.full(totcols * 128, PHANTOM, dtype=np.int16)
        stream[tokpos] = srow_s.astype(np.int16)
        # pack [128, totcols*8]: per call contiguous [16, ntok/16] tiled x8
        blocks = []
        for g in range(NGRP):
            for q in range(NQ):
                cb = callbase[g, q]
                blk = stream[cb * 128 : (cb + cols[g, q]) * 128]
                blocks.append(np.tile(blk.reshape(-1, 16).T, (8, 1)))
        idx_streams.append(np.ascontiguousarray(np.concatenate(blocks, axis=1)))

    return {
        "orders": orders,
        "nodes": nodes_c,
        "Kq": Kq,
        "cols": cols,
        "totcols": int(totcols),
        "idx": idx_streams,
    }


def _apx(base: AP, off: int, dims) -> AP:
    """AP with base's partition dim and explicit free [step, count] dims."""
    return AP(base.tensor, base.offset + off, [list(base.ap[0])] + [list(d) for d in dims])


# ---------------------------------------------------------------- device build
def _build(Kq, cols, totcols):
    Kq = [[int(v) for v in row] for row in Kq]
    cols = [[int(v) for v in row] for row in cols]
    KQMAX = max(max(r) for r in Kq)
    GCMAX = max(max(r) for r in cols)
    TOT16 = totcols * 8

    nc = bacc.Bacc("TRN2", target_bir_lowering=False, debug=False, num_devices=NCORES)

    xT = nc.dram_tensor("xT", [F_IN, NL], BF16, kind="ExternalInput")
    w1 = nc.dram_tensor("w1", [F_IN, D1], BF16, kind="ExternalInput")
    w2 = nc.dram_tensor("w2", [D1, C], BF16, kind="ExternalInput")
    cvec = nc.dram_tensor("cvec", [128, 6 * 64], F32, kind="ExternalInput")
    negd = nc.dram_tensor("negd", [NL - NLR, ELEM], BF16, kind="ExternalInput")
    idxd = nc.dram_tensor("idxd", [128, TOT16], I16, kind="ExternalInput")
    outd = nc.dram_tensor("outv", [NL, C], F32, kind="ExternalOutput")

    t1b = nc.dram_tensor("t1b", [NL, ELEM], BF16)
    T1 = nc.dram_tensor("T1", [GT, ELEM], BF16, addr_space="Shared")
    t2b = nc.dram_tensor("t2b", [NL, ELEM], BF16)
    T2 = nc.dram_tensor("T2", [GT, ELEM], BF16, addr_space="Shared")

    with tile.TileContext(nc) as tc:
        with (
            tc.tile_pool(name="consts", bufs=1) as cpool,
            tc.tile_pool(name="persist", bufs=1) as ppool,
            tc.tile_pool(name="xload", bufs=3) as xpool,
            tc.tile_pool(name="stg", bufs=3) as stgpool,
            tc.tile_pool(name="ipool", bufs=4) as ipool,
            tc.tile_pool(name="gpool", bufs=4) as gpool,
            tc.tile_pool(name="zpool", bufs=4) as zpool,
            tc.tile_pool(name="small", bufs=4) as spool,
            tc.tile_pool(name="psum", bufs=4, space="PSUM") as pspool,
        ):
            # ---- constants
            w1sb = cpool.tile([128, 4 * D1], BF16)
            nc.sync.dma_start(
                out=w1sb[:].rearrange("p (cc d) -> p cc d", cc=4),
                in_=w1[:, :].rearrange("(cc p) d -> p cc d", p=128),
            )
            w2sb = cpool.tile([128, C], BF16)
            nc.sync.dma_start(out=w2sb[:D1, :], in_=w2[:, :])
            cv = cpool.tile([128, 6 * 64], F32)
            nc.sync.dma_start(out=cv[:], in_=cvec[:, :])
            asrs = cv[:, 0:64]
            adss = cv[:, 64:128]
            a2ss = cv[:, 128:192]
            a2ds = cv[:, 192:256]
            b1s = cv[:, 256:320]
            b2s = cv[:, 320:384]
            ident = cpool.tile([128, 128], F32)
            make_identity(nc, ident[:])

            # ---- persistent
            x2st = ppool.tile([128, NWIN * D1], F32)
            edt = ppool.tile([128, NWIN * H1 + NWIN], F32)
            dent = ppool.tile([128, NWIN * H1 + NWIN], F32)

            def node_phase(layer):
                tb, Tg = (t1b, T1) if layer == 1 else (t2b, T2)
                for sb in range(0, NWIN, SB):
                    stg = stgpool.tile([128, SB * ELEM], BF16, tag="stg")
                    nc.vector.memset(stg[:], 0.0)
                    for w in range(sb, sb + SB):
                        wl = w - sb
                        if layer == 1 and w % XB == 0:
                            xb = xpool.tile([128, 4 * XB * 128], BF16, tag="xb")
                            nc.sync.dma_start(
                                out=xb[:].rearrange("p (cc n) -> p cc n", cc=4),
                                in_=xT[:, w * 128 : (w + XB) * 128].rearrange(
                                    "(cc p) n -> p cc n", p=128
                                ),
                            )
                        ph = pspool.tile([128, D1], F32, tag="ph")
                        if layer == 1:
                            nn = XB * 128
                            for cc in range(4):
                                nc.tensor.matmul(
                                    out=ph[:],
                                    lhsT=_apx(xb[:], cc * nn + (w % XB) * 128, [[1, 128]]),
                                    rhs=_apx(w1sb[:], cc * D1, [[1, D1]]),
                                    start=(cc == 0),
                                    stop=(cc == 3),
                                )
                        else:
                            pt = pspool.tile([64, 128], F32, tag="pt")
                            nc.tensor.transpose(
                                out=pt[:],
                                in_=_apx(x2st[:], w * D1, [[1, D1]]),
                                identity=ident[:],
                            )
                            x1t = spool.tile([64, 128], BF16, tag="x1t")
                            nc.vector.tensor_copy(out=x1t[:], in_=pt[:])
                            nc.tensor.matmul(
                                out=ph[:], lhsT=x1t[:], rhs=w2sb[:D1, :],
                                start=True, stop=True,
                            )
                        hcol = _apx(stg[:], wl * ELEM, [[1, D1]])
                        nc.vector.tensor_copy(out=hcol, in_=ph[:])
                        a_s = asrs if layer == 1 else a2ss
                        a_d = adss if layer == 1 else a2ds
                        tmp = spool.tile([128, 2 * D1], F32, tag="tmp")
                        nc.vector.tensor_tensor(out=tmp[:, :D1], in0=ph[:], in1=a_s, op=OP.mult)
                        nc.vector.tensor_tensor(out=tmp[:, D1:], in0=ph[:], in1=a_d, op=OP.mult)
                        if layer == 1:
                            with nc.allow_low_precision(reason="es row is bf16 by design"):
                                nc.vector.tensor_reduce(
                                    out=_apx(stg[:], wl * ELEM + D1, [[1, H1]]),
                                    in_=_apx(tmp[:], 0, [[FH1, H1], [1, FH1]]),
                                    axis=AX, op=OP.add)
                            nc.vector.tensor_reduce(
                                out=_apx(edt[:], w * H1, [[1, H1]]),
                                in_=_apx(tmp[:], D1, [[FH1, H1], [1, FH1]]),
                                axis=AX, op=OP.add)
                        else:
                            with nc.allow_low_precision(reason="es row is bf16 by design"):
                                nc.vector.tensor_reduce(
                                    out=_apx(stg[:], wl * ELEM + D1, [[1, 1]]),
                                    in_=_apx(tmp[:], 0, [[1, C]]),
                                    axis=AX, op=OP.add)
                            nc.vector.tensor_reduce(
                                out=_apx(edt[:], NWIN * H1 + w, [[1, 1]]),
                                in_=_apx(tmp[:], D1, [[1, C]]),
                                axis=AX, op=OP.add)
                    nc.sync.dma_start(
                        out=tb[sb * 128 : (sb + SB) * 128, :].rearrange(
                            "(w p) r -> p w r", p=128
                        ),
                        in_=stg[:].rearrange("p (w r) -> p w r", w=SB),
                    )
                nc.sync.dma_start(out=tb[NLR:NL, :], in_=negd[:, :])
                nc.gpsimd.collective_compute(
                    "AllGather", OP.bypass,
                    replica_groups=[list(range(NCORES))],
                    ins=[tb[:, :]], outs=[Tg[:, :]],
                )

            def edge_phase(layer):
                Tg = T1 if layer == 1 else T2
                doff = 0 if layer == 1 else NWIN * H1
                nc.vector.memset(x2st[:], 0.0)
                if layer == 1:
                    nc.vector.memset(dent[:, : NWIN * H1], 0.0)
                else:
                    nc.vector.memset(dent[:, NWIN * H1 :], 0.0)
                off16 = 0
                for g in range(NGRP):
                    for q in range(NQ):
                        ncols = cols[g][q]
                        ntok = ncols * 128
                        it = ipool.tile([128, GCMAX * 8], I16, tag="it")
                        nc.sync.dma_start(
                            out=it[:, : ntok // 16],
                            in_=idxd[:, off16 : off16 + ntok // 16],
                        )
                        G = gpool.tile([128, GCMAX * ELEM], BF16, tag="G")
                        CH = TOKCHUNK // 128
                        for c0 in range(0, ncols, CH):
                            cc = min(CH, ncols - c0)
                            nc.gpsimd.dma_gather(
                                out_ap=G[:, c0 * ELEM : (c0 + cc) * ELEM].rearrange(
                                    "p (b e) -> p b e", e=ELEM
                                ),
                                in_ap=Tg[q * QR : (q + 1) * QR, :],
                                idxs_ap=it[:, c0 * 8 : (c0 + cc) * 8],
                                num_idxs=cc * 128,
                                num_idxs_reg=cc * 128,
                                elem_size=ELEM,
                                single_packet=True,
                            )
                        cw = 0
                        for w in range(g * GW, (g + 1) * GW):
                            Kw = Kq[w][q]
                            base = cw * ELEM
                            cw += Kw
                            if Kw == 0:
                                continue
                            dn = spool.tile([128, 64], F32, tag="dn")
                            if layer == 1:
                                z = zpool.tile([128, H1 * KQMAX], F32, tag="z")
                                nc.vector.tensor_tensor(
                                    out=_apx(z[:], 0, [[Kw, H1], [1, Kw]]),
                                    in0=_apx(G[:], base + D1, [[1, H1], [ELEM, Kw]]),
                                    in1=_apx(edt[:], w * H1, [[1, H1], [0, Kw]]),
                                    op=OP.add)
                                zf = _apx(z[:], 0, [[1, H1 * Kw]])
                                nc.vector.scalar_tensor_tensor(
                                    out=zf, in0=zf, scalar=0.2, in1=zf,
                                    op0=OP.mult, op1=OP.max)
                                zb = zpool.tile([128, H1 * KQMAX], BF16, tag="zb")
                                zbf = _apx(zb[:], 0, [[1, H1 * Kw]])
                                nc.scalar.activation(out=zbf, in_=zf, func=AF.Exp)
                                nc.vector.tensor_reduce(
                                    out=dn[:, 0:H1],
                                    in_=_apx(zb[:], 0, [[Kw, H1], [1, Kw]]),
                                    axis=AX, op=OP.add)
                                dch = _apx(dent[:], w * H1, [[1, H1]])
                                nc.vector.tensor_tensor(
                                    out=dch, in0=dch, in1=dn[:, 0:H1], op=OP.add)
                                # weighted products packed k-major (contiguous),
                                # then pairwise tree-halving: avoids 256B-strided
                                # reduce reads
                                pk = zpool.tile([128, KQMAX * D1], F32, tag="pk")
                                nc.vector.tensor_tensor(
                                    out=_apx(pk[:], 0, [[D1, Kw], [FH1, H1], [1, FH1]]),
                                    in0=_apx(G[:], base, [[ELEM, Kw], [FH1, H1], [1, FH1]]),
                                    in1=_apx(zb[:], 0, [[1, Kw], [Kw, H1], [0, FH1]]),
                                    op=OP.mult)
                                cur = Kw
                                while cur > 1:
                                    half = cur // 2
                                    lo = _apx(pk[:], 0, [[1, half * D1]])
                                    hi = _apx(pk[:], (cur - half) * D1, [[1, half * D1]])
                                    nc.vector.tensor_tensor(
                                        out=lo, in0=lo, in1=hi, op=OP.add)
                                    cur -= half
                                xch = _apx(x2st[:], w * D1, [[1, D1]])
                                nc.vector.tensor_tensor(
                                    out=xch, in0=xch, in1=_apx(pk[:], 0, [[1, D1]]),
                                    op=OP.add)
                            else:
                                z = zpool.tile([128, H1 * KQMAX], F32, tag="z")
                                nc.vector.tensor_tensor(
                                    out=_apx(z[:], 0, [[1, Kw]]),
                                    in0=_apx(G[:], base + D1, [[ELEM, Kw]]),
                                    in1=_apx(edt[:], NWIN * H1 + w, [[0, Kw]]),
                                    op=OP.add)
                                zf = _apx(z[:], 0, [[1, Kw]])
                                nc.vector.scalar_tensor_tensor(
                                    out=zf, in0=zf, scalar=0.2, in1=zf,
                                    op0=OP.mult, op1=OP.max)
                                zb = zpool.tile([128, H1 * KQMAX], BF16, tag="zb")
                                zbf = _apx(zb[:], 0, [[1, Kw]])
                                nc.scalar.activation(out=zbf, in_=zf, func=AF.Exp)
                                nc.vector.tensor_reduce(
                                    out=dn[:, 0:1], in_=zbf, axis=AX, op=OP.add)
                                dch = _apx(dent[:], NWIN * H1 + w, [[1, 1]])
                                nc.vector.tensor_tensor(
                                    out=dch, in0=dch, in1=dn[:, 0:1], op=OP.add)
                                pk = zpool.tile([128, KQMAX * D1], F32, tag="pk")
                                nc.vector.tensor_tensor(
                                    out=_apx(pk[:], 0, [[C, Kw], [1, C]]),
                                    in0=_apx(G[:], base, [[ELEM, Kw], [1, C]]),
                                    in1=_apx(zb[:], 0, [[1, Kw], [0, C]]),
                                    op=OP.mult)
                                cur = Kw
                                while cur > 1:
                                    half = cur // 2
                                    lo = _apx(pk[:], 0, [[1, half * C]])
                                    hi = _apx(pk[:], (cur - half) * C, [[1, half * C]])
                                    nc.vector.tensor_tensor(
                                        out=lo, in0=lo, in1=hi, op=OP.add)
                                    cur -= half
                                xch = _apx(x2st[:], w * C, [[1, C]])
                                nc.vector.tensor_tensor(
                                    out=xch, in0=xch, in1=_apx(pk[:], 0, [[1, C]]),
                                    op=OP.add)
                        off16 += ntok // 16

            # ================= layer 1 =================
            node_phase(1)
            edge_phase(1)
            # x1 = elu(num/den + b1), chunked
            for gch in range(0, NWIN, SB):
                dch = _apx(dent[:], gch * H1, [[1, SB * H1]])
                nc.vector.tensor_scalar_add(dch, dch, 1e-30)
                rec = spool.tile([128, SB * H1], F32, tag="rec")
                nc.vector.reciprocal(out=rec[:], in_=dch)
                xs = _apx(x2st[:], gch * D1, [[1, SB * D1]])
                nc.vector.tensor_tensor(
                    out=_apx(x2st[:], gch * D1, [[D1, SB], [FH1, H1], [1, FH1]]),
                    in0=_apx(x2st[:], gch * D1, [[D1, SB], [FH1, H1], [1, FH1]]),
                    in1=_apx(rec[:], 0, [[H1, SB], [1, H1], [0, FH1]]),
                    op=OP.mult)
                nc.vector.tensor_tensor(
                    out=xs, in0=xs, in1=_apx(b1s, 0, [[0, SB], [1, D1]]), op=OP.add)
                tmp = spool.tile([128, SB * D1], F32, tag="tail")
                tf = _apx(tmp[:], 0, [[1, SB * D1]])
                nc.vector.tensor_scalar_min(tf, xs, 0.0)
                nc.scalar.activation(out=tf, in_=tf, func=AF.Exp)
                nc.vector.tensor_scalar_max(xs, xs, 0.0)
                nc.vector.scalar_tensor_tensor(
                    out=xs, in0=tf, scalar=-1.0, in1=xs, op0=OP.add, op1=OP.add)

            # ================= layer 2 =================
            node_phase(2)
            edge_phase(2)
            # out = log_softmax(num/den + b2), chunked
            for gch in range(0, NWIN, SB):
                dch = _apx(dent[:], NWIN * H1 + gch, [[1, SB]])
                nc.vector.tensor_scalar_add(dch, dch, 1e-30)
                rec = spool.tile([128, SB * H1], F32, tag="rec")
                nc.vector.reciprocal(out=rec[:, :SB], in_=dch)
                xs = _apx(x2st[:], gch * C, [[1, SB * C]])
                nc.vector.tensor_tensor(
                    out=_apx(x2st[:], gch * C, [[C, SB], [1, C]]),
                    in0=_apx(x2st[:], gch * C, [[C, SB], [1, C]]),
                    in1=_apx(rec[:], 0, [[1, SB], [0, C]]),
                    op=OP.mult)
                nc.vector.tensor_tensor(
                    out=xs, in0=xs, in1=_apx(b2s, 0, [[0, SB], [1, C]]), op=OP.add)
                rmx = spool.tile([128, SB], F32, tag="rmx")
                nc.vector.tensor_reduce(
                    out=rmx[:], in_=_apx(x2st[:], gch * C, [[C, SB], [1, C]]),
                    axis=AX, op=OP.max)
                nc.vector.tensor_tensor(
                    out=xs, in0=xs, in1=_apx(rmx[:], 0, [[1, SB], [0, C]]),
                    op=OP.subtract)
                tmp = spool.tile([128, SB * C], F32, tag="tail")
                tf = _apx(tmp[:], 0, [[1, SB * C]])
                nc.scalar.activation(out=tf, in_=xs, func=AF.Exp)
                nc.vector.tensor_reduce(
                    out=rmx[:], in_=_apx(tmp[:], 0, [[C, SB], [1, C]]),
                    axis=AX, op=OP.add)
                nc.scalar.activation(out=rmx[:], in_=rmx[:], func=AF.Ln)
                nc.vector.tensor_tensor(
                    out=xs, in0=xs, in1=_apx(rmx[:], 0, [[1, SB], [0, C]]),
                    op=OP.subtract)
            nc.sync.dma_start(
                out=outd[:, :].rearrange("(w p) f -> p w f", p=128),
                in_=x2st[:].rearrange("p (w f) -> p w f", w=NWIN),
            )

    nc.compile()
    return nc


# ---------------------------------------------------------------- PJRT runner
def _make_runner(nc):
    import jax
    from jax.sharding import Mesh, PartitionSpec, NamedSharding
    from jax.experimental.shard_map import shard_map
    from concourse import bass2jax
    from concourse.bass2jax import _bass_exec_p, install_neuronx_cc_hook

    install_neuronx_cc_hook()
    partition_name = nc.partition_id_tensor.name if nc.partition_id_tensor else None
    in_names, out_names, out_avals = [], [], []
    for alloc in nc.m.functions[0].allocations:
        if not isinstance(alloc, mybir.MemoryLocationSet):
            continue
        name = alloc.memorylocations[0].name
        if alloc.kind == "ExternalInput":
            if name != partition_name:
                in_names.append(name)
        elif alloc.kind == "ExternalOutput":
            out_avals.append(
                jax.core.ShapedArray(tuple(alloc.tensor_shape), mybir.dt.np(alloc.dtype))
            )
            out_names.append(name)
    n_params = len(in_names)
    all_in = list(in_names) + list(out_names)
    if partition_name is not None:
        all_in.append(partition_name)

    def _body(*args):
        operands = list(args)
        if partition_name is not None:
            operands.append(bass2jax.partition_id_tensor())
        return tuple(
            _bass_exec_p.bind(
                *operands,
                out_avals=tuple(out_avals),
                in_names=tuple(all_in),
                out_names=tuple(out_names),
                lowering_input_output_aliases=(),
                sim_require_finite=True,
                sim_require_nnan=True,
                nc=nc,
            )
        )

    devices = jax.devices()[:NCORES]
    mesh = Mesh(np.asarray(devices), ("core",))
    n_outs = len(out_names)
    sharded = jax.jit(
        shard_map(
            _body, mesh=mesh,
            in_specs=(PartitionSpec("core"),) * (n_params + n_outs),
            out_specs=(PartitionSpec("core"),) * n_outs,
            check_rep=False,
        ),
        keep_unused=True,
    )
    sharding = NamedSharding(mesh, PartitionSpec("core"))

    def run(in_maps):
        import jax as _jax

        per_core = [[np.asarray(m[nm]) for nm in in_names] for m in in_maps]
        concat_in = [
            np.concatenate([per_core[c][i] for c in range(NCORES)], axis=0)
            for i in range(n_params)
        ]
        concat_zero = [
            np.zeros((NCORES * a.shape[0], *a.shape[1:]), a.dtype) for a in out_avals
        ]
        args = [_jax.device_put(x, sharding) for x in concat_in + concat_zero]
        out = sharded(*args)
        _jax.block_until_ready(out)
        return (
            [
                {
                    nm: np.asarray(out[i]).reshape(NCORES, *out_avals[i].shape)[c]
                    for i, nm in enumerate(out_names)
                }
                for c in range(NCORES)
            ],
            sharded,
            args,
        )

    return run


_CACHE = {}


def _get_compiled(Kq, cols, totcols):
    key = (totcols, tuple(tuple(int(v) for v in r) for r in Kq))
    if key not in _CACHE:
        nc = _build(Kq, cols, totcols)
        _CACHE[key] = (nc, _make_runner(nc))
    return _CACHE[key]


def _prep_inputs(x, plan, W1, att1_src, att1_dst, b1, W2, att2_src, att2_dst, b2):
    import ml_dtypes

    bf16 = ml_dtypes.bfloat16
    cvec = np.zeros((128, 6 * 64), np.float32)
    cvec[:, 0:64] = np.asarray(att1_src, np.float32).reshape(1, D1)
    cvec[:, 64:128] = np.asarray(att1_dst, np.float32).reshape(1, D1)
    cvec[:, 128:192] = np.asarray(att2_src, np.float32).reshape(1, C)
    cvec[:, 192:256] = np.asarray(att2_dst, np.float32).reshape(1, C)
    cvec[:, 256:320] = np.asarray(b1, np.float32).reshape(1, D1)
    cvec[:, 320:384] = np.asarray(b2, np.float32).reshape(1, C)
    w1b = np.ascontiguousarray(np.asarray(W1, np.float32)).astype(bf16)
    w2b = np.ascontiguousarray(np.asarray(W2, np.float32)).astype(bf16)
    negv = np.full((NL - NLR, ELEM), NEG, np.float32).astype(bf16)
    in_maps = []
    for c in range(NCORES):
        order = plan["orders"][c]
        xp = np.zeros((NL, F_IN), np.float32)
        xp[:NLR] = x[plan["nodes"][c]][order]
        in_maps.append(
            {
                "xT": np.ascontiguousarray(xp.T).astype(bf16),
                "w1": w1b,
                "w2": w2b,
                "cvec": cvec,
                "negd": negv,
                "idxd": plan["idx"][c],
            }
        )
    return in_maps


def kernel(x, edge_index, W1, att1_src, att1_dst, b1, W2, att2_src, att2_dst, b2):
    x = np.asarray(x, np.float32)
    edge_index = np.asarray(edge_index)
    plan = _plan(edge_index)
    nc, run = _get_compiled(plan["Kq"], plan["cols"], plan["totcols"])
    in_maps = _prep_inputs(
        x, plan,
        np.asarray(W1), np.asarray(att1_src), np.asarray(att1_dst), np.asarray(b1),
        np.asarray(W2), np.asarray(att2_src), np.asarray(att2_dst), np.asarray(b2),
    )
    results, _, _ = run(in_maps)
    out = np.empty((N, C), np.float32)
    for c in range(NCORES):
        out[plan["nodes"][c][plan["orders"][c]]] = results[c]["outv"][:NLR]
    return out
